# revision 13
# baseline (speedup 1.0000x reference)
"""AdaMoLE (LoRA-MoE routing) Trainium2 kernel, data-parallel over tokens on 8 cores.

Math (per token n):
    logits = x @ Wr.T + br                 [E]
    gate   = softmax(logits)
    thr    = sigmoid(x @ Wt.T + bt)        [1]
    w      = relu(8*gate - thr); w /= max(sum(w), eps)   (scale-invariant vs ref)
    h      = x @ A_all                     [E*R]
    out    = (h * rep(w)) @ (B_all * SCALING)

Layout: x is STATIONARY in mm1 — one fused pass per 128-d chunk computes both
h and the 9 routing logits into one [128tok, 137] psum tile (the baseline
streamed x through PE twice).  Routing math runs in token-partition layout on
ACT/DVE with free-dim reductions; the routing weights are applied with a
single stride-0 broadcast multiply.  hw is PE-transposed to [er, tok] for mm2.
I/O is bf16 both ways (host casts), halving DMA volume vs f32.
"""

import sys

sys.path.insert(0, "/opt/trn_rl_repo")

import numpy as np
import ml_dtypes

import concourse.bacc as bacc
import concourse.mybir as mybir
import concourse.tile as tile
from concourse.bass_utils import run_bass_kernel_spmd
from contextlib import ExitStack

F32 = mybir.dt.float32
BF16 = mybir.dt.bfloat16
AF = mybir.ActivationFunctionType

B, S, D, DOUT = 4, 4096, 4096, 4096
R, E, SCALING = 16, 8, 8.0 / 16
NCORES = 8
N = B * S
NTOK = N // NCORES        # 2048 tokens per core
BS = 128                  # tokens per block (= stationary width)
NBLK = NTOK // BS         # 16
NDC = D // 128            # 32 contraction chunks
ER = E * R                # 128
CW = ER + 9               # fused mm1 output width (h ++ router logits)
NOB = DOUT // 512         # 8 output column blocks

_CACHE = {}


XGS = [1, 1, 2, 4, 4, 4]  # X DMA group sizes in blocks: small first groups
                          # start mm1 ~9us earlier; big tail groups keep
                          # 32KB/partition lines for bandwidth
OGRP = 2                  # blocks per OUT DMA (2 MB transfers, 16 KB lines)

# Best-known build config (applied by kernel() and the timing harness).
# Per-block X DMAs keep PE streaming instead of stalling ~14us on 4-block
# group boundaries; single-shot sim: 112.8us -> 103.0us (bus floor).
BEST_KW = {"xgs": [1] * NBLK}


def _build(reps=1, loop=False, do_compute=True, do_dma=True,
           x_rings=("sync",), o_rings=("scalar",), body_reps=1,
           with_bias=False, xgs=None, ogrp=None, awc_split=1,
           ph_bufs=2, po_bufs=5, ocopy_flip=False, staggered=False,
           tail_split=0):
    XGS = list(xgs) if xgs is not None else globals()["XGS"]
    OGRP = ogrp if ogrp is not None else globals()["OGRP"]
    assert sum(XGS) == NBLK and NBLK % OGRP == 0
    nc = bacc.Bacc("TRN2", debug=False, num_devices=NCORES)

    # X and OUT are partition-contiguous so DMA lines are 32KB/16KB per
    # partition (1MB-per-block layouts cap at ~78% of HBM bandwidth)
    X = nc.declare_dram_parameter("X", [128, NBLK * NDC * BS], BF16, isOutput=False)
    AWc = nc.declare_dram_parameter("AWc", [128, NDC * CW], BF16, isOutput=False)
    BIASR = nc.declare_dram_parameter("BIASR", [1, CW], BF16, isOutput=False)
    IDN = nc.declare_dram_parameter("IDN", [128, 128], BF16, isOutput=False)
    Bl = nc.declare_dram_parameter("Bl", [ER, DOUT], BF16, isOutput=False)
    OUT = nc.declare_dram_parameter("out", [128, NBLK * DOUT], BF16, isOutput=True)

    with tile.TileContext(nc) as tc, ExitStack() as ctx:
        wpool = ctx.enter_context(tc.tile_pool(name="w", bufs=1))
        # all X groups resident: X DMAs never wait on compute within a rep
        xpool = ctx.enter_context(tc.tile_pool(name="x", bufs=1))
        opool = ctx.enter_context(tc.tile_pool(name="o", bufs=3))
        spool = ctx.enter_context(tc.tile_pool(name="s", bufs=2))
        hwpool = ctx.enter_context(tc.tile_pool(name="hw", bufs=2))
        # 2+1+5 = all 8 banks: pt=1 is safe (the hwT copy completes an
        # iteration before the next transpose reuses the bank); 5 po banks
        # decouple mm2's rotation from the copy engines, which also lets
        # ps buffers free early enough that ph=2 costs no mm1 run-ahead
        ph = ctx.enter_context(tc.tile_pool(name="ph", bufs=ph_bufs, space="PSUM"))
        pt = ctx.enter_context(tc.tile_pool(name="pt", bufs=1, space="PSUM"))
        po = ctx.enter_context(tc.tile_pool(name="po", bufs=po_bufs, space="PSUM"))

        # weights on the scalar ring (stores ring, idle at start) so X can
        # start streaming on the sync ring immediately.  awc_split>1 loads the
        # first d-chunks as a separate small DMA so mm1(block 0) can start as
        # soon as ~35KB of AWc has landed instead of the full 1.12MB
        if awc_split > 1:
            AWc0_sb = wpool.tile([128, awc_split * CW], BF16, tag="AWc0")
            nc.scalar.dma_start(out=AWc0_sb[:], in_=AWc[:, : awc_split * CW])
            AWcR_sb = wpool.tile([128, (NDC - awc_split) * CW], BF16, tag="AWcR")
            nc.scalar.dma_start(out=AWcR_sb[:], in_=AWc[:, awc_split * CW :])

            def awc_slice(dc):
                if dc < awc_split:
                    return AWc0_sb[:, dc * CW : (dc + 1) * CW]
                return AWcR_sb[:, (dc - awc_split) * CW : (dc - awc_split + 1) * CW]
        else:
            AWc_sb = wpool.tile([128, NDC * CW], BF16, tag="AWc")
            nc.scalar.dma_start(out=AWc_sb[:], in_=AWc[:])

            def awc_slice(dc):
                return AWc_sb[:, dc * CW : (dc + 1) * CW]
        biasr_sb = wpool.tile([1, CW], BF16, tag="biasr")
        nc.scalar.dma_start(out=biasr_sb[:], in_=BIASR[:])
        idn_sb = wpool.tile([128, 128], BF16, tag="IDN")
        nc.scalar.dma_start(out=idn_sb[:], in_=IDN[:])
        B_sb = wpool.tile([ER, DOUT], BF16, tag="B")
        nc.scalar.dma_start(out=B_sb[:], in_=Bl[:])
        ones1 = wpool.tile([1, 128], BF16, tag="ones1")
        nc.vector.memset(ones1[:], 1.0)

        def eng(name):
            return getattr(nc, name)

        BLKC = NDC * BS  # 4096 cols per block
        XOFF = [sum(XGS[:i]) for i in range(len(XGS))]  # first block of group i

        def emit_all():
            xg = []
            for g, gs in enumerate(XGS):
                xt = xpool.tile([128, gs * BLKC], BF16, tag=f"xg{g}")
                if do_dma:
                    ring = x_rings[g % len(x_rings)]
                    eng(ring).dma_start(
                        out=xt[:],
                        in_=X[:, XOFF[g] * BLKC : (XOFF[g] + gs) * BLKC],
                    )
                elif g == 0:
                    nc.vector.memset(xt[:], 0.01)
                xg.append(xt)

            def xslice(b, c0, c1):
                if not do_dma:
                    return xg[0][:, c0:c1]
                g = max(i for i in range(len(XGS)) if XOFF[i] <= b)
                off = (b - XOFF[g]) * BLKC
                return xg[g][:, off + c0 : off + c1]

            ps_l = [None] * NBLK
            hw_l = [None] * NBLK
            hwT_l = [None] * NBLK

            def stA(b):  # mm1: fused h + routing logits, x stationary
                ps = ph.tile([128, CW], F32, tag="ps")
                for dc in range(NDC):
                    nc.tensor.matmul(
                        ps[:],
                        xslice(b, dc * BS, (dc + 1) * BS),
                        awc_slice(dc),
                        start=(dc == 0),
                        stop=(not with_bias and dc == NDC - 1),
                    )
                if with_bias:
                    # rank-1 bias add via K=1 matmul; skipped when br/bt are
                    # all zero (the build is specialized on the input values)
                    nc.tensor.matmul(
                        ps[:], ones1[:], biasr_sb[:], start=False, stop=True
                    )
                ps_l[b] = ps

            def stB(b):  # routing math + broadcast-weight multiply
                ps = ps_l[b]
                eexp = spool.tile([128, E], F32, tag="eexp")
                S1 = spool.tile([128, 1], F32, tag="S1")
                nc.scalar.activation(eexp[:], ps[:, ER : ER + E], AF.Exp, accum_out=S1[:])
                # sigmoid via exp so ACT stays on one act-func set (no
                # 1.3us LUT reloads): thr/E = 1 / (E * (1 + exp(-z)))
                texp = spool.tile([128, 1], F32, tag="texp")
                nc.scalar.activation(texp[:], ps[:, ER + E : ER + E + 1], AF.Exp, scale=-1.0)
                u = spool.tile([128, 1], F32, tag="u")
                nc.vector.tensor_scalar(
                    u[:], texp[:], 1.0, float(E),
                    mybir.AluOpType.add, mybir.AluOpType.mult,
                )
                thr8 = spool.tile([128, 1], F32, tag="thr8")
                nc.vector.reciprocal(thr8[:], u[:])
                sg1 = spool.tile([128, 1], F32, tag="sg1")
                nc.vector.reciprocal(sg1[:], S1[:])
                adapted = spool.tile([128, E], F32, tag="ad")
                nc.vector.tensor_scalar(
                    adapted[:], eexp[:], sg1[:], thr8[:],
                    mybir.AluOpType.mult, mybir.AluOpType.subtract,
                )
                # relu with a +1.25e-31 floor: S2 = sum >= 1e-30, so the
                # reciprocal below is NaN-safe without a separate max op
                # (mathematically some expert is always selected: max gate
                # >= 1/E >= thr/E, so the floor never alters real outputs)
                wrel = spool.tile([128, E], F32, tag="wr")
                S2 = spool.tile([128, 1], F32, tag="S2")
                nc.vector.tensor_scalar(
                    wrel[:], adapted[:], 0.0, 1.25e-31,
                    mybir.AluOpType.max, mybir.AluOpType.add, accum_out=S2[:],
                )
                sr = spool.tile([128, 1], F32, tag="sr")
                nc.vector.reciprocal(sr[:], S2[:])
                wfin = spool.tile([128, E], F32, tag="wf")
                nc.vector.tensor_scalar_mul(wfin[:], wrel[:], sr[:])
                hw_t = hwpool.tile([128, ER], BF16, tag="hwt")
                nc.vector.tensor_mul(
                    hw_t[:].rearrange("p (e r) -> p e r", e=E),
                    ps[:, 0:ER].rearrange("p (e r) -> p e r", e=E),
                    wfin[:].unsqueeze(2).broadcast_to([128, E, R]),
                )
                hw_l[b] = hw_t

            def stC(b):  # PE transpose [tok, er] -> [er, tok]
                tp = pt.tile([128, 128], BF16, tag="tp")
                nc.tensor.transpose(tp[:], hw_l[b][:], idn_sb[:])
                hwT_l[b] = tp

            def stD(b):  # psum -> sbuf for mm2 stationary (on ACT: keeps the
                # DVE queue free between the routing chain and the o-copies)
                hs = hwpool.tile([128, 128], BF16, tag="hwT")
                nc.scalar.activation(hs[:], hwT_l[b][:], AF.Copy)
                hwT_l[b] = hs

            o_cur = [None]

            def stEFG(b):  # mm2 + staged copies + grouped store
                if b % OGRP == 0:
                    o_new = opool.tile([128, OGRP * DOUT], BF16, tag="osb")
                    o_cur[0] = o_new
                o_sb = o_cur[0]
                ob = (b % OGRP) * DOUT
                for nb in range(NOB):
                    o_ps = po.tile([128, 512], F32, tag="o")
                    nc.tensor.matmul(
                        o_ps[:],
                        hwT_l[b][:],
                        B_sb[:, nb * 512 : (nb + 1) * 512],
                        start=True,
                        stop=True,
                    )
                    # alternate engines so consecutive psum->sbuf copies run
                    # in parallel and mm2's 3-bank rotation is never paced by
                    # one engine's serial copy chain
                    if (nb % 2 == 0) != ocopy_flip:
                        nc.scalar.activation(
                            o_sb[:, ob + nb * 512 : ob + (nb + 1) * 512], o_ps[:], AF.Copy
                        )
                    else:
                        nc.vector.tensor_copy(
                            o_sb[:, ob + nb * 512 : ob + (nb + 1) * 512], o_ps[:]
                        )
                if do_dma:
                    g = b // OGRP
                    ring = o_rings[g % len(o_rings)]
                    if tail_split and b >= NBLK - tail_split:
                        # stream the last block(s) out in per-512-col pieces
                        # right after each psum->sbuf copy lands, so the
                        # final DMA is 128KB instead of 1MB
                        for nb in range(NOB):
                            eng(ring).dma_start(
                                out=OUT[:, b * DOUT + nb * 512 : b * DOUT + (nb + 1) * 512],
                                in_=o_sb[:, ob + nb * 512 : ob + (nb + 1) * 512],
                            )
                    elif b >= NBLK - OGRP:
                        # final group: per-block 1MB stores so the tail
                        # drains as soon as each block's copies land
                        eng(ring).dma_start(
                            out=OUT[:, b * DOUT : (b + 1) * DOUT],
                            in_=o_sb[:, ob : ob + DOUT],
                        )
                    elif b % OGRP == OGRP - 1:
                        eng(ring).dma_start(
                            out=OUT[:, g * OGRP * DOUT : (g + 1) * OGRP * DOUT],
                            in_=o_sb[:],
                        )

            if not do_compute:
                # DMA-only ablation: consume each X group with a 1-col matmul
                # (so loads stay on the critical path), store a constant o_sb
                o_sb = wpool.tile([128, OGRP * DOUT], BF16, tag="osbc")
                nc.gpsimd.memset(o_sb[:], 0.02)
                for g in range(len(XGS)):
                    acc = po.tile([128, 1], F32, tag="acc")
                    nc.tensor.matmul(
                        acc[:], xg[g][:, 0:128], xg[g][:, 0:1], start=True, stop=True
                    )
                for g in range(NBLK // OGRP):
                    if do_dma:
                        ring = o_rings[g % len(o_rings)]
                        eng(ring).dma_start(
                            out=OUT[:, g * OGRP * DOUT : (g + 1) * OGRP * DOUT],
                            in_=o_sb[:],
                        )
                return

            # 2-deep software pipeline so PE never waits on the ACT/DVE
            # routing chain or the hwT copy.  stB(b+2) is emitted LAST: its
            # exp sem-waits on mm1(b+2), and ACT/DVE process in order, so
            # putting it before the copies/store of block b would head-of-line
            # block the store stream whenever PE is DMA-paced.
            stA(0); stB(0)
            stA(1); stB(1)
            stC(0); stD(0)
            for b in range(NBLK):
                if b + 2 < NBLK:
                    stA(b + 2)
                stEFG(b)
                if b + 1 < NBLK:
                    stC(b + 1); stD(b + 1)
                if b + 2 < NBLK:
                    stB(b + 2)

        if loop:
            with tc.For_i(0, reps, 1, staggered_reset=staggered):
                for _ in range(body_reps):
                    emit_all()
        else:
            for _ in range(reps):
                emit_all()

    nc.compile()
    return nc


def _prep_consts(Wr, br, Wt, bt, A, Bw):
    bf = ml_dtypes.bfloat16
    A_all = np.asarray(A, np.float32).transpose(1, 0, 2).reshape(D, ER)  # [d, er]
    Wcat = np.concatenate(
        [np.asarray(Wr, np.float32).T, np.asarray(Wt, np.float32).T], axis=1
    )  # [d, 9]
    AWc_h = np.concatenate(
        [A_all.reshape(NDC, 128, ER), Wcat.reshape(NDC, 128, 9)], axis=2
    )  # [NDC, 128, CW]
    AWc_host = np.ascontiguousarray(
        AWc_h.transpose(1, 0, 2).reshape(128, NDC * CW)
    ).astype(bf)
    biasr = np.zeros((1, CW), np.float32)
    biasr[0, ER : ER + E] = np.asarray(br, np.float32)
    biasr[0, ER + E] = np.float32(np.asarray(bt).reshape(()))
    B_host = (np.asarray(Bw, np.float32).reshape(ER, DOUT) * SCALING).astype(bf)
    idn = np.eye(128, dtype=np.float32).astype(bf)
    return {
        "AWc": AWc_host,
        "BIASR": biasr.astype(bf),
        "IDN": idn,
        "Bl": B_host,
    }


def _prep_x(xs):
    """Per-core shard [NTOK, D] -> [128, NBLK*NDC*BS] bf16,
    partition-contiguous: partition p holds [blk, dc, t] so each X DMA reads
    groups of 8KB-per-block contiguous per partition."""
    arr = (
        np.asarray(xs, np.float32)
        .reshape(NBLK, BS, NDC, 128)
        .transpose(3, 0, 2, 1)  # [p, blk, dc, t]
        .reshape(128, NBLK * NDC * BS)
    )
    return np.ascontiguousarray(arr).astype(ml_dtypes.bfloat16)


def kernel(x, Wr, br, Wt, bt, A, Bw, _trace=False, _trace_kwargs=None):
    # specialize the build on whether the router/threshold biases are zero
    # (they are in the reference); nonzero biases take the with_bias path
    wb = bool(np.any(np.asarray(br)) or np.any(np.asarray(bt)))
    key = f"nc{int(wb)}"
    if key not in _CACHE:
        _CACHE[key] = _build(with_bias=wb, **BEST_KW)
    nc = _CACHE[key]

    consts = _prep_consts(Wr, br, Wt, bt, A, Bw)
    xf = np.asarray(x, np.float32).reshape(N, D)
    in_maps = []
    for c in range(NCORES):
        Xh = _prep_x(xf[c * NTOK : (c + 1) * NTOK])
        in_maps.append({"X": Xh, **consts})

    res = run_bass_kernel_spmd(
        nc,
        in_maps,
        core_ids=list(range(NCORES)),
        trace=_trace,
        **(_trace_kwargs or {}),
    )
    # OUT dram layout is [128, NBLK*DOUT] (partition p = token b*128+p)
    out = np.concatenate(
        [
            np.asarray(res.results[c]["out"], np.float32)
            .reshape(128, NBLK, DOUT)
            .transpose(1, 0, 2)
            .reshape(NTOK, DOUT)
            for c in range(NCORES)
        ],
        axis=0,
    )
    if _trace:
        _CACHE["last_res"] = res
    return out.reshape(B, S, DOUT).astype(np.float32)



# revision 26
# speedup vs baseline: 1.0123x; 1.0123x over previous
"""AdaMoLE (LoRA-MoE routing) Trainium2 kernel, data-parallel over tokens on 8 cores.

Math (per token n):
    logits = x @ Wr.T + br                 [E]
    gate   = softmax(logits)
    thr    = sigmoid(x @ Wt.T + bt)        [1]
    w      = relu(8*gate - thr); w /= max(sum(w), eps)   (scale-invariant vs ref)
    h      = x @ A_all                     [E*R]
    out    = (h * rep(w)) @ (B_all * SCALING)

Layout: x is STATIONARY in mm1 — one fused pass per 128-d chunk computes both
h and the 9 routing logits into one [128tok, 137] psum tile (the baseline
streamed x through PE twice).  Routing math runs in token-partition layout on
ACT/DVE with free-dim reductions; the routing weights are applied with a
single stride-0 broadcast multiply.  hw is PE-transposed to [er, tok] for mm2.
I/O is bf16 both ways (host casts), halving DMA volume vs f32.
"""

import sys

sys.path.insert(0, "/opt/trn_rl_repo")

import numpy as np
import ml_dtypes

import concourse.bacc as bacc
import concourse.mybir as mybir
import concourse.tile as tile
from concourse.bass_utils import run_bass_kernel_spmd
from contextlib import ExitStack

F32 = mybir.dt.float32
BF16 = mybir.dt.bfloat16
AF = mybir.ActivationFunctionType

B, S, D, DOUT = 4, 4096, 4096, 4096
R, E, SCALING = 16, 8, 8.0 / 16
NCORES = 8
N = B * S
NTOK = N // NCORES        # 2048 tokens per core
BS = 128                  # tokens per block (= stationary width)
NBLK = NTOK // BS         # 16
NDC = D // 128            # 32 contraction chunks
ER = E * R                # 128
CW = ER + 9               # fused mm1 output width (h ++ router logits)
NOB = DOUT // 512         # 8 output column blocks

_CACHE = {}


XGS = [1, 1, 2, 4, 4, 4]  # X DMA group sizes in blocks: small first groups
                          # start mm1 ~9us earlier; big tail groups keep
                          # 32KB/partition lines for bandwidth
OGRP = 2                  # blocks per OUT DMA (2 MB transfers, 16 KB lines)

# Best-known build config (applied by kernel() and the timing harness).
# Per-block X DMAs keep PE streaming instead of stalling ~14us on 4-block
# group boundaries (single-shot sim: 112.8us -> 103.0us, the bus floor);
# wpack=2 merges the IDN/B constant loads into one DMA behind the
# mm1-critical AWc load; alternating OUT store groups across the scalar
# and sync DMA rings hides per-DMA descriptor-gen dead time (HW: 113.1 ->
# 105.4-106.8us loop-marginal, and it also wins in the continuous
# body_reps=2 regime, 108.7 vs 112.0 per rep).
BEST_KW = {"xgs": [1] * NBLK, "wpack": 2, "o_rings": ("scalar", "sync")}


def _build(reps=1, loop=False, do_compute=True, do_dma=True,
           x_rings=("sync",), o_rings=("scalar",), body_reps=1,
           with_bias=False, xgs=None, ogrp=None, awc_split=1,
           ph_bufs=2, po_bufs=5, ocopy_flip=False, staggered=False,
           tail_split=0, wpack=0):
    XGS = list(xgs) if xgs is not None else globals()["XGS"]
    OGRP = ogrp if ogrp is not None else globals()["OGRP"]
    assert sum(XGS) == NBLK and NBLK % OGRP == 0
    nc = bacc.Bacc("TRN2", debug=False, num_devices=NCORES)

    # X and OUT are partition-contiguous so DMA lines are 32KB/16KB per
    # partition (1MB-per-block layouts cap at ~78% of HBM bandwidth)
    X = nc.declare_dram_parameter("X", [128, NBLK * NDC * BS], BF16, isOutput=False)
    if wpack:
        # all constants in one dram tensor: [AWc | IDN | B] -> 1-2 DMAs
        WP = nc.declare_dram_parameter(
            "WPACK", [128, NDC * CW + 128 + DOUT], BF16, isOutput=False
        )
    else:
        AWc = nc.declare_dram_parameter("AWc", [128, NDC * CW], BF16, isOutput=False)
        IDN = nc.declare_dram_parameter("IDN", [128, 128], BF16, isOutput=False)
        Bl = nc.declare_dram_parameter("Bl", [ER, DOUT], BF16, isOutput=False)
    if with_bias:
        BIASR = nc.declare_dram_parameter("BIASR", [1, CW], BF16, isOutput=False)
    OUT = nc.declare_dram_parameter("out", [128, NBLK * DOUT], BF16, isOutput=True)

    with tile.TileContext(nc) as tc, ExitStack() as ctx:
        wpool = ctx.enter_context(tc.tile_pool(name="w", bufs=1))
        # all X groups resident: X DMAs never wait on compute within a rep
        xpool = ctx.enter_context(tc.tile_pool(name="x", bufs=1))
        opool = ctx.enter_context(tc.tile_pool(name="o", bufs=3))
        spool = ctx.enter_context(tc.tile_pool(name="s", bufs=2))
        hwpool = ctx.enter_context(tc.tile_pool(name="hw", bufs=2))
        # 2+1+5 = all 8 banks: pt=1 is safe (the hwT copy completes an
        # iteration before the next transpose reuses the bank); 5 po banks
        # decouple mm2's rotation from the copy engines, which also lets
        # ps buffers free early enough that ph=2 costs no mm1 run-ahead
        ph = ctx.enter_context(tc.tile_pool(name="ph", bufs=ph_bufs, space="PSUM"))
        pt = ctx.enter_context(tc.tile_pool(name="pt", bufs=1, space="PSUM"))
        po = ctx.enter_context(tc.tile_pool(name="po", bufs=po_bufs, space="PSUM"))

        # weights on the scalar ring (stores ring, idle at start) so X can
        # start streaming on the sync ring immediately.
        NWC = NDC * CW
        if wpack:
            wp_sb = wpool.tile([128, NWC + 128 + DOUT], BF16, tag="WP")
            if wpack == 1:
                # single DMA for all constants
                nc.scalar.dma_start(out=wp_sb[:], in_=WP[:])
            else:
                # AWc first (mm1-critical), then IDN|B in one DMA
                nc.scalar.dma_start(out=wp_sb[:, 0:NWC], in_=WP[:, 0:NWC])
                nc.scalar.dma_start(out=wp_sb[:, NWC:], in_=WP[:, NWC:])

            def awc_slice(dc):
                return wp_sb[:, dc * CW : (dc + 1) * CW]

            def idn_ap():
                return wp_sb[:, NWC : NWC + 128]

            def b_slice(c0, c1):
                return wp_sb[:, NWC + 128 + c0 : NWC + 128 + c1]
        else:
            if awc_split > 1:
                AWc0_sb = wpool.tile([128, awc_split * CW], BF16, tag="AWc0")
                nc.scalar.dma_start(out=AWc0_sb[:], in_=AWc[:, : awc_split * CW])
                AWcR_sb = wpool.tile([128, (NDC - awc_split) * CW], BF16, tag="AWcR")
                nc.scalar.dma_start(out=AWcR_sb[:], in_=AWc[:, awc_split * CW :])

                def awc_slice(dc):
                    if dc < awc_split:
                        return AWc0_sb[:, dc * CW : (dc + 1) * CW]
                    return AWcR_sb[
                        :, (dc - awc_split) * CW : (dc - awc_split + 1) * CW
                    ]
            else:
                AWcF_sb = wpool.tile([128, NWC], BF16, tag="AWc")
                nc.scalar.dma_start(out=AWcF_sb[:], in_=AWc[:])

                def awc_slice(dc):
                    return AWcF_sb[:, dc * CW : (dc + 1) * CW]

            idnT = wpool.tile([128, 128], BF16, tag="IDN")
            nc.scalar.dma_start(out=idnT[:], in_=IDN[:])
            BT = wpool.tile([ER, DOUT], BF16, tag="B")
            nc.scalar.dma_start(out=BT[:], in_=Bl[:])

            def idn_ap():
                return idnT[:]

            def b_slice(c0, c1):
                return BT[:, c0:c1]

        if with_bias:
            biasr_sb = wpool.tile([1, CW], BF16, tag="biasr")
            nc.scalar.dma_start(out=biasr_sb[:], in_=BIASR[:])
            ones1 = wpool.tile([1, 128], BF16, tag="ones1")
            nc.vector.memset(ones1[:], 1.0)

        def eng(name):
            return getattr(nc, name)

        BLKC = NDC * BS  # 4096 cols per block
        XOFF = [sum(XGS[:i]) for i in range(len(XGS))]  # first block of group i

        def emit_all():
            xg = []
            for g, gs in enumerate(XGS):
                xt = xpool.tile([128, gs * BLKC], BF16, tag=f"xg{g}")
                if do_dma:
                    ring = x_rings[g % len(x_rings)]
                    eng(ring).dma_start(
                        out=xt[:],
                        in_=X[:, XOFF[g] * BLKC : (XOFF[g] + gs) * BLKC],
                    )
                elif g == 0:
                    nc.vector.memset(xt[:], 0.01)
                xg.append(xt)

            def xslice(b, c0, c1):
                if not do_dma:
                    return xg[0][:, c0:c1]
                g = max(i for i in range(len(XGS)) if XOFF[i] <= b)
                off = (b - XOFF[g]) * BLKC
                return xg[g][:, off + c0 : off + c1]

            ps_l = [None] * NBLK
            hw_l = [None] * NBLK
            hwT_l = [None] * NBLK

            def stA(b):  # mm1: fused h + routing logits, x stationary
                ps = ph.tile([128, CW], F32, tag="ps")
                for dc in range(NDC):
                    nc.tensor.matmul(
                        ps[:],
                        xslice(b, dc * BS, (dc + 1) * BS),
                        awc_slice(dc),
                        start=(dc == 0),
                        stop=(not with_bias and dc == NDC - 1),
                    )
                if with_bias:
                    # rank-1 bias add via K=1 matmul; skipped when br/bt are
                    # all zero (the build is specialized on the input values)
                    nc.tensor.matmul(
                        ps[:], ones1[:], biasr_sb[:], start=False, stop=True
                    )
                ps_l[b] = ps

            def stB(b):  # routing math + broadcast-weight multiply
                ps = ps_l[b]
                eexp = spool.tile([128, E], F32, tag="eexp")
                S1 = spool.tile([128, 1], F32, tag="S1")
                nc.scalar.activation(eexp[:], ps[:, ER : ER + E], AF.Exp, accum_out=S1[:])
                # sigmoid via exp so ACT stays on one act-func set (no
                # 1.3us LUT reloads): thr/E = 1 / (E * (1 + exp(-z)))
                texp = spool.tile([128, 1], F32, tag="texp")
                nc.scalar.activation(texp[:], ps[:, ER + E : ER + E + 1], AF.Exp, scale=-1.0)
                u = spool.tile([128, 1], F32, tag="u")
                nc.vector.tensor_scalar(
                    u[:], texp[:], 1.0, float(E),
                    mybir.AluOpType.add, mybir.AluOpType.mult,
                )
                thr8 = spool.tile([128, 1], F32, tag="thr8")
                nc.vector.reciprocal(thr8[:], u[:])
                sg1 = spool.tile([128, 1], F32, tag="sg1")
                nc.vector.reciprocal(sg1[:], S1[:])
                adapted = spool.tile([128, E], F32, tag="ad")
                nc.vector.tensor_scalar(
                    adapted[:], eexp[:], sg1[:], thr8[:],
                    mybir.AluOpType.mult, mybir.AluOpType.subtract,
                )
                # relu with a +1.25e-31 floor: S2 = sum >= 1e-30, so the
                # reciprocal below is NaN-safe without a separate max op
                # (mathematically some expert is always selected: max gate
                # >= 1/E >= thr/E, so the floor never alters real outputs)
                wrel = spool.tile([128, E], F32, tag="wr")
                S2 = spool.tile([128, 1], F32, tag="S2")
                nc.vector.tensor_scalar(
                    wrel[:], adapted[:], 0.0, 1.25e-31,
                    mybir.AluOpType.max, mybir.AluOpType.add, accum_out=S2[:],
                )
                sr = spool.tile([128, 1], F32, tag="sr")
                nc.vector.reciprocal(sr[:], S2[:])
                wfin = spool.tile([128, E], F32, tag="wf")
                nc.vector.tensor_scalar_mul(wfin[:], wrel[:], sr[:])
                hw_t = hwpool.tile([128, ER], BF16, tag="hwt")
                nc.vector.tensor_mul(
                    hw_t[:].rearrange("p (e r) -> p e r", e=E),
                    ps[:, 0:ER].rearrange("p (e r) -> p e r", e=E),
                    wfin[:].unsqueeze(2).broadcast_to([128, E, R]),
                )
                hw_l[b] = hw_t

            def stC(b):  # PE transpose [tok, er] -> [er, tok]
                tp = pt.tile([128, 128], BF16, tag="tp")
                nc.tensor.transpose(tp[:], hw_l[b][:], idn_ap())
                hwT_l[b] = tp

            def stD(b):  # psum -> sbuf for mm2 stationary (on ACT: keeps the
                # DVE queue free between the routing chain and the o-copies)
                hs = hwpool.tile([128, 128], BF16, tag="hwT")
                nc.scalar.activation(hs[:], hwT_l[b][:], AF.Copy)
                hwT_l[b] = hs

            o_cur = [None]

            def stEFG(b):  # mm2 + staged copies + grouped store
                if b % OGRP == 0:
                    o_new = opool.tile([128, OGRP * DOUT], BF16, tag="osb")
                    o_cur[0] = o_new
                o_sb = o_cur[0]
                ob = (b % OGRP) * DOUT
                for nb in range(NOB):
                    o_ps = po.tile([128, 512], F32, tag="o")
                    nc.tensor.matmul(
                        o_ps[:],
                        hwT_l[b][:],
                        b_slice(nb * 512, (nb + 1) * 512),
                        start=True,
                        stop=True,
                    )
                    # alternate engines so consecutive psum->sbuf copies run
                    # in parallel and mm2's 3-bank rotation is never paced by
                    # one engine's serial copy chain
                    if (nb % 2 == 0) != ocopy_flip:
                        nc.scalar.activation(
                            o_sb[:, ob + nb * 512 : ob + (nb + 1) * 512], o_ps[:], AF.Copy
                        )
                    else:
                        nc.vector.tensor_copy(
                            o_sb[:, ob + nb * 512 : ob + (nb + 1) * 512], o_ps[:]
                        )
                if do_dma:
                    g = b // OGRP
                    ring = o_rings[g % len(o_rings)]
                    if tail_split and b >= NBLK - tail_split:
                        # stream the last block(s) out in per-512-col pieces
                        # right after each psum->sbuf copy lands, so the
                        # final DMA is 128KB instead of 1MB
                        for nb in range(NOB):
                            eng(ring).dma_start(
                                out=OUT[:, b * DOUT + nb * 512 : b * DOUT + (nb + 1) * 512],
                                in_=o_sb[:, ob + nb * 512 : ob + (nb + 1) * 512],
                            )
                    elif b >= NBLK - OGRP:
                        # final group: per-block 1MB stores so the tail
                        # drains as soon as each block's copies land; ring
                        # alternates by block so the two drain stores can
                        # overlap their DGE setup across rings
                        eng(o_rings[b % len(o_rings)]).dma_start(
                            out=OUT[:, b * DOUT : (b + 1) * DOUT],
                            in_=o_sb[:, ob : ob + DOUT],
                        )
                    elif b % OGRP == OGRP - 1:
                        eng(ring).dma_start(
                            out=OUT[:, g * OGRP * DOUT : (g + 1) * OGRP * DOUT],
                            in_=o_sb[:],
                        )

            if not do_compute:
                # DMA-only ablation: consume each X group with a 1-col matmul
                # (so loads stay on the critical path), store a constant o_sb
                o_sb = wpool.tile([128, OGRP * DOUT], BF16, tag="osbc")
                nc.gpsimd.memset(o_sb[:], 0.02)
                for g in range(len(XGS)):
                    acc = po.tile([128, 1], F32, tag="acc")
                    nc.tensor.matmul(
                        acc[:], xg[g][:, 0:128], xg[g][:, 0:1], start=True, stop=True
                    )
                for g in range(NBLK // OGRP):
                    if do_dma:
                        ring = o_rings[g % len(o_rings)]
                        eng(ring).dma_start(
                            out=OUT[:, g * OGRP * DOUT : (g + 1) * OGRP * DOUT],
                            in_=o_sb[:],
                        )
                return

            # 2-deep software pipeline so PE never waits on the ACT/DVE
            # routing chain or the hwT copy.  stB(b+2) is emitted LAST: its
            # exp sem-waits on mm1(b+2), and ACT/DVE process in order, so
            # putting it before the copies/store of block b would head-of-line
            # block the store stream whenever PE is DMA-paced.
            stA(0); stB(0)
            stA(1); stB(1)
            stC(0); stD(0)
            for b in range(NBLK):
                if b + 2 < NBLK:
                    stA(b + 2)
                stEFG(b)
                if b + 1 < NBLK:
                    stC(b + 1); stD(b + 1)
                if b + 2 < NBLK:
                    stB(b + 2)
                # staggered==2: explicit 4-block-aligned stage boundaries so
                # the staggered sem reset never cuts mid-block (the auto
                # instruction-count split misaligns engines and serializes)
                if staggered == 2 and loop and b in (3, 7, 11):
                    tc.stage_boundary()

        if loop:
            assert not (staggered == 2 and body_reps != 1)
            with tc.For_i(0, reps, 1, staggered_reset=bool(staggered)):
                for _ in range(body_reps):
                    emit_all()
        else:
            for _ in range(reps):
                emit_all()

    nc.compile()
    return nc


def _prep_consts(Wr, br, Wt, bt, A, Bw):
    bf = ml_dtypes.bfloat16
    A_all = np.asarray(A, np.float32).transpose(1, 0, 2).reshape(D, ER)  # [d, er]
    Wcat = np.concatenate(
        [np.asarray(Wr, np.float32).T, np.asarray(Wt, np.float32).T], axis=1
    )  # [d, 9]
    AWc_h = np.concatenate(
        [A_all.reshape(NDC, 128, ER), Wcat.reshape(NDC, 128, 9)], axis=2
    )  # [NDC, 128, CW]
    AWc_host = np.ascontiguousarray(
        AWc_h.transpose(1, 0, 2).reshape(128, NDC * CW)
    ).astype(bf)
    biasr = np.zeros((1, CW), np.float32)
    biasr[0, ER : ER + E] = np.asarray(br, np.float32)
    biasr[0, ER + E] = np.float32(np.asarray(bt).reshape(()))
    B_host = (np.asarray(Bw, np.float32).reshape(ER, DOUT) * SCALING).astype(bf)
    idn = np.eye(128, dtype=np.float32).astype(bf)
    wpack_host = np.ascontiguousarray(
        np.concatenate([AWc_host, idn, B_host], axis=1)
    )
    return {
        "AWc": AWc_host,
        "BIASR": biasr.astype(bf),
        "IDN": idn,
        "Bl": B_host,
        "WPACK": wpack_host,
    }


def _prep_x(xs):
    """Per-core shard [NTOK, D] -> [128, NBLK*NDC*BS] bf16,
    partition-contiguous: partition p holds [blk, dc, t] so each X DMA reads
    groups of 8KB-per-block contiguous per partition."""
    arr = (
        np.asarray(xs, np.float32)
        .reshape(NBLK, BS, NDC, 128)
        .transpose(3, 0, 2, 1)  # [p, blk, dc, t]
        .reshape(128, NBLK * NDC * BS)
    )
    return np.ascontiguousarray(arr).astype(ml_dtypes.bfloat16)


def kernel(x, Wr, br, Wt, bt, A, Bw, _trace=False, _trace_kwargs=None):
    # specialize the build on whether the router/threshold biases are zero
    # (they are in the reference); nonzero biases take the with_bias path
    wb = bool(np.any(np.asarray(br)) or np.any(np.asarray(bt)))
    key = f"nc{int(wb)}"
    if key not in _CACHE:
        _CACHE[key] = _build(with_bias=wb, **BEST_KW)
    nc = _CACHE[key]

    consts = _prep_consts(Wr, br, Wt, bt, A, Bw)
    xf = np.asarray(x, np.float32).reshape(N, D)
    in_maps = []
    for c in range(NCORES):
        Xh = _prep_x(xf[c * NTOK : (c + 1) * NTOK])
        in_maps.append({"X": Xh, **consts})

    res = run_bass_kernel_spmd(
        nc,
        in_maps,
        core_ids=list(range(NCORES)),
        trace=_trace,
        **(_trace_kwargs or {}),
    )
    # OUT dram layout is [128, NBLK*DOUT] (partition p = token b*128+p)
    out = np.concatenate(
        [
            np.asarray(res.results[c]["out"], np.float32)
            .reshape(128, NBLK, DOUT)
            .transpose(1, 0, 2)
            .reshape(NTOK, DOUT)
            for c in range(NCORES)
        ],
        axis=0,
    )
    if _trace:
        _CACHE["last_res"] = res
    return out.reshape(B, S, DOUT).astype(np.float32)



# revision 27
# speedup vs baseline: 1.0336x; 1.0210x over previous
"""AdaMoLE (LoRA-MoE routing) Trainium2 kernel, data-parallel over tokens on 8 cores.

Math (per token n):
    logits = x @ Wr.T + br                 [E]
    gate   = softmax(logits)
    thr    = sigmoid(x @ Wt.T + bt)        [1]
    w      = relu(8*gate - thr); w /= max(sum(w), eps)   (scale-invariant vs ref)
    h      = x @ A_all                     [E*R]
    out    = (h * rep(w)) @ (B_all * SCALING)

Layout: x is STATIONARY in mm1 — one fused pass per 128-d chunk computes both
h and the 9 routing logits into one [128tok, 137] psum tile (the baseline
streamed x through PE twice).  Routing math runs in token-partition layout on
ACT/DVE with free-dim reductions; the routing weights are applied with a
single stride-0 broadcast multiply.  hw is PE-transposed to [er, tok] for mm2.
I/O is bf16 both ways (host casts), halving DMA volume vs f32.
"""

import sys

sys.path.insert(0, "/opt/trn_rl_repo")

import numpy as np
import ml_dtypes

import concourse.bacc as bacc
import concourse.mybir as mybir
import concourse.tile as tile
from concourse.bass_utils import run_bass_kernel_spmd
from contextlib import ExitStack

F32 = mybir.dt.float32
BF16 = mybir.dt.bfloat16
AF = mybir.ActivationFunctionType

B, S, D, DOUT = 4, 4096, 4096, 4096
R, E, SCALING = 16, 8, 8.0 / 16
NCORES = 8
N = B * S
NTOK = N // NCORES        # 2048 tokens per core
BS = 128                  # tokens per block (= stationary width)
NBLK = NTOK // BS         # 16
NDC = D // 128            # 32 contraction chunks
ER = E * R                # 128
CW = ER + 9               # fused mm1 output width (h ++ router logits)
NOB = DOUT // 512         # 8 output column blocks

_CACHE = {}


XGS = [1, 1, 2, 4, 4, 4]  # X DMA group sizes in blocks: small first groups
                          # start mm1 ~9us earlier; big tail groups keep
                          # 32KB/partition lines for bandwidth
OGRP = 2                  # blocks per OUT DMA (2 MB transfers, 16 KB lines)

# Best-known build config (applied by kernel() and the timing harness).
# Per-block X DMAs keep PE streaming instead of stalling ~14us on 4-block
# group boundaries (single-shot sim: 112.8us -> 103.0us, the bus floor);
# wpack=2 merges the IDN/B constant loads into one DMA behind the
# mm1-critical AWc load; alternating OUT store groups across the scalar
# and sync DMA rings hides per-DMA descriptor-gen dead time (HW: 113.1 ->
# 105.4-106.8us loop-marginal, and it also wins in the continuous
# body_reps=2 regime, 108.7 vs 112.0 per rep).
BEST_KW = {"xgs": [1] * NBLK, "wpack": 2, "o_rings": ("scalar", "sync")}


def _build(reps=1, loop=False, do_compute=True, do_dma=True,
           x_rings=("sync",), o_rings=("scalar",), body_reps=1,
           with_bias=False, xgs=None, ogrp=None, awc_split=1,
           ph_bufs=2, po_bufs=5, ocopy_flip=False, staggered=False,
           tail_split=0, wpack=0):
    XGS = list(xgs) if xgs is not None else globals()["XGS"]
    OGRP = ogrp if ogrp is not None else globals()["OGRP"]
    assert sum(XGS) == NBLK and NBLK % OGRP == 0
    nc = bacc.Bacc("TRN2", debug=False, num_devices=NCORES)

    # X and OUT are partition-contiguous so DMA lines are 32KB/16KB per
    # partition (1MB-per-block layouts cap at ~78% of HBM bandwidth)
    X = nc.declare_dram_parameter("X", [128, NBLK * NDC * BS], BF16, isOutput=False)
    if wpack:
        # all constants in one dram tensor: [AWc | IDN | B] -> 1-2 DMAs
        WP = nc.declare_dram_parameter(
            "WPACK", [128, NDC * CW + 128 + DOUT], BF16, isOutput=False
        )
    else:
        AWc = nc.declare_dram_parameter("AWc", [128, NDC * CW], BF16, isOutput=False)
        IDN = nc.declare_dram_parameter("IDN", [128, 128], BF16, isOutput=False)
        Bl = nc.declare_dram_parameter("Bl", [ER, DOUT], BF16, isOutput=False)
    if with_bias:
        BIASR = nc.declare_dram_parameter("BIASR", [1, CW], BF16, isOutput=False)
    OUT = nc.declare_dram_parameter("out", [128, NBLK * DOUT], BF16, isOutput=True)

    with tile.TileContext(nc) as tc, ExitStack() as ctx:
        wpool = ctx.enter_context(tc.tile_pool(name="w", bufs=1))
        # all X groups resident: X DMAs never wait on compute within a rep
        xpool = ctx.enter_context(tc.tile_pool(name="x", bufs=1))
        opool = ctx.enter_context(tc.tile_pool(name="o", bufs=3))
        spool = ctx.enter_context(tc.tile_pool(name="s", bufs=2))
        hwpool = ctx.enter_context(tc.tile_pool(name="hw", bufs=2))
        # 2+1+5 = all 8 banks: pt=1 is safe (the hwT copy completes an
        # iteration before the next transpose reuses the bank); 5 po banks
        # decouple mm2's rotation from the copy engines, which also lets
        # ps buffers free early enough that ph=2 costs no mm1 run-ahead
        ph = ctx.enter_context(tc.tile_pool(name="ph", bufs=ph_bufs, space="PSUM"))
        pt = ctx.enter_context(tc.tile_pool(name="pt", bufs=1, space="PSUM"))
        po = ctx.enter_context(tc.tile_pool(name="po", bufs=po_bufs, space="PSUM"))

        # weights on the scalar ring (stores ring, idle at start) so X can
        # start streaming on the sync ring immediately.
        NWC = NDC * CW
        if wpack:
            wp_sb = wpool.tile([128, NWC + 128 + DOUT], BF16, tag="WP")
            if wpack == 1:
                # single DMA for all constants
                nc.scalar.dma_start(out=wp_sb[:], in_=WP[:])
            else:
                # AWc first (mm1-critical), then IDN|B in one DMA
                nc.scalar.dma_start(out=wp_sb[:, 0:NWC], in_=WP[:, 0:NWC])
                nc.scalar.dma_start(out=wp_sb[:, NWC:], in_=WP[:, NWC:])

            def awc_slice(dc):
                return wp_sb[:, dc * CW : (dc + 1) * CW]

            def idn_ap():
                return wp_sb[:, NWC : NWC + 128]

            def b_slice(c0, c1):
                return wp_sb[:, NWC + 128 + c0 : NWC + 128 + c1]
        else:
            if awc_split > 1:
                AWc0_sb = wpool.tile([128, awc_split * CW], BF16, tag="AWc0")
                nc.scalar.dma_start(out=AWc0_sb[:], in_=AWc[:, : awc_split * CW])
                AWcR_sb = wpool.tile([128, (NDC - awc_split) * CW], BF16, tag="AWcR")
                nc.scalar.dma_start(out=AWcR_sb[:], in_=AWc[:, awc_split * CW :])

                def awc_slice(dc):
                    if dc < awc_split:
                        return AWc0_sb[:, dc * CW : (dc + 1) * CW]
                    return AWcR_sb[
                        :, (dc - awc_split) * CW : (dc - awc_split + 1) * CW
                    ]
            else:
                AWcF_sb = wpool.tile([128, NWC], BF16, tag="AWc")
                nc.scalar.dma_start(out=AWcF_sb[:], in_=AWc[:])

                def awc_slice(dc):
                    return AWcF_sb[:, dc * CW : (dc + 1) * CW]

            idnT = wpool.tile([128, 128], BF16, tag="IDN")
            nc.scalar.dma_start(out=idnT[:], in_=IDN[:])
            BT = wpool.tile([ER, DOUT], BF16, tag="B")
            nc.scalar.dma_start(out=BT[:], in_=Bl[:])

            def idn_ap():
                return idnT[:]

            def b_slice(c0, c1):
                return BT[:, c0:c1]

        if with_bias:
            biasr_sb = wpool.tile([1, CW], BF16, tag="biasr")
            nc.scalar.dma_start(out=biasr_sb[:], in_=BIASR[:])
            ones1 = wpool.tile([1, 128], BF16, tag="ones1")
            nc.vector.memset(ones1[:], 1.0)

        def eng(name):
            return getattr(nc, name)

        BLKC = NDC * BS  # 4096 cols per block
        XOFF = [sum(XGS[:i]) for i in range(len(XGS))]  # first block of group i

        def emit_all():
            xg = []
            for g, gs in enumerate(XGS):
                xt = xpool.tile([128, gs * BLKC], BF16, tag=f"xg{g}")
                if do_dma:
                    ring = x_rings[g % len(x_rings)]
                    eng(ring).dma_start(
                        out=xt[:],
                        in_=X[:, XOFF[g] * BLKC : (XOFF[g] + gs) * BLKC],
                    )
                elif g == 0:
                    nc.vector.memset(xt[:], 0.01)
                xg.append(xt)

            def xslice(b, c0, c1):
                if not do_dma:
                    return xg[0][:, c0:c1]
                g = max(i for i in range(len(XGS)) if XOFF[i] <= b)
                off = (b - XOFF[g]) * BLKC
                return xg[g][:, off + c0 : off + c1]

            ps_l = [None] * NBLK
            hw_l = [None] * NBLK
            hwT_l = [None] * NBLK

            def stA(b):  # mm1: fused h + routing logits, x stationary
                ps = ph.tile([128, CW], F32, tag="ps")
                for dc in range(NDC):
                    nc.tensor.matmul(
                        ps[:],
                        xslice(b, dc * BS, (dc + 1) * BS),
                        awc_slice(dc),
                        start=(dc == 0),
                        stop=(not with_bias and dc == NDC - 1),
                    )
                if with_bias:
                    # rank-1 bias add via K=1 matmul; skipped when br/bt are
                    # all zero (the build is specialized on the input values)
                    nc.tensor.matmul(
                        ps[:], ones1[:], biasr_sb[:], start=False, stop=True
                    )
                ps_l[b] = ps

            def stB(b):  # routing math + broadcast-weight multiply
                ps = ps_l[b]
                eexp = spool.tile([128, E], F32, tag="eexp")
                S1 = spool.tile([128, 1], F32, tag="S1")
                nc.scalar.activation(eexp[:], ps[:, ER : ER + E], AF.Exp, accum_out=S1[:])
                # sigmoid via exp so ACT stays on one act-func set (no
                # 1.3us LUT reloads): thr/E = 1 / (E * (1 + exp(-z)))
                texp = spool.tile([128, 1], F32, tag="texp")
                nc.scalar.activation(texp[:], ps[:, ER + E : ER + E + 1], AF.Exp, scale=-1.0)
                u = spool.tile([128, 1], F32, tag="u")
                nc.vector.tensor_scalar(
                    u[:], texp[:], 1.0, float(E),
                    mybir.AluOpType.add, mybir.AluOpType.mult,
                )
                thr8 = spool.tile([128, 1], F32, tag="thr8")
                nc.vector.reciprocal(thr8[:], u[:])
                sg1 = spool.tile([128, 1], F32, tag="sg1")
                nc.vector.reciprocal(sg1[:], S1[:])
                adapted = spool.tile([128, E], F32, tag="ad")
                nc.vector.tensor_scalar(
                    adapted[:], eexp[:], sg1[:], thr8[:],
                    mybir.AluOpType.mult, mybir.AluOpType.subtract,
                )
                # relu with a +1.25e-31 floor: S2 = sum >= 1e-30, so the
                # reciprocal below is NaN-safe without a separate max op
                # (mathematically some expert is always selected: max gate
                # >= 1/E >= thr/E, so the floor never alters real outputs)
                wrel = spool.tile([128, E], F32, tag="wr")
                S2 = spool.tile([128, 1], F32, tag="S2")
                nc.vector.tensor_scalar(
                    wrel[:], adapted[:], 0.0, 1.25e-31,
                    mybir.AluOpType.max, mybir.AluOpType.add, accum_out=S2[:],
                )
                sr = spool.tile([128, 1], F32, tag="sr")
                nc.vector.reciprocal(sr[:], S2[:])
                wfin = spool.tile([128, E], F32, tag="wf")
                nc.vector.tensor_scalar_mul(wfin[:], wrel[:], sr[:])
                hw_t = hwpool.tile([128, ER], BF16, tag="hwt")
                nc.vector.tensor_mul(
                    hw_t[:].rearrange("p (e r) -> p e r", e=E),
                    ps[:, 0:ER].rearrange("p (e r) -> p e r", e=E),
                    wfin[:].unsqueeze(2).broadcast_to([128, E, R]),
                )
                hw_l[b] = hw_t

            def stC(b):  # PE transpose [tok, er] -> [er, tok]
                tp = pt.tile([128, 128], BF16, tag="tp")
                nc.tensor.transpose(tp[:], hw_l[b][:], idn_ap())
                hwT_l[b] = tp

            def stD(b):  # psum -> sbuf for mm2 stationary (on ACT: keeps the
                # DVE queue free between the routing chain and the o-copies)
                hs = hwpool.tile([128, 128], BF16, tag="hwT")
                nc.scalar.activation(hs[:], hwT_l[b][:], AF.Copy)
                hwT_l[b] = hs

            o_cur = [None]

            def stEFG(b):  # mm2 + staged copies + grouped store
                if b % OGRP == 0:
                    o_new = opool.tile([128, OGRP * DOUT], BF16, tag="osb")
                    o_cur[0] = o_new
                o_sb = o_cur[0]
                ob = (b % OGRP) * DOUT
                for nb in range(NOB):
                    o_ps = po.tile([128, 512], F32, tag="o")
                    nc.tensor.matmul(
                        o_ps[:],
                        hwT_l[b][:],
                        b_slice(nb * 512, (nb + 1) * 512),
                        start=True,
                        stop=True,
                    )
                    # alternate engines so consecutive psum->sbuf copies run
                    # in parallel and mm2's 3-bank rotation is never paced by
                    # one engine's serial copy chain
                    if (nb % 2 == 0) != ocopy_flip:
                        nc.scalar.activation(
                            o_sb[:, ob + nb * 512 : ob + (nb + 1) * 512], o_ps[:], AF.Copy
                        )
                    else:
                        nc.vector.tensor_copy(
                            o_sb[:, ob + nb * 512 : ob + (nb + 1) * 512], o_ps[:]
                        )
                if do_dma:
                    g = b // OGRP
                    ring = o_rings[g % len(o_rings)]
                    if tail_split and b >= NBLK - tail_split:
                        # stream the last block(s) out in per-512-col pieces
                        # right after each psum->sbuf copy lands, so the
                        # final DMA is 128KB instead of 1MB
                        for nb in range(NOB):
                            eng(ring).dma_start(
                                out=OUT[:, b * DOUT + nb * 512 : b * DOUT + (nb + 1) * 512],
                                in_=o_sb[:, ob + nb * 512 : ob + (nb + 1) * 512],
                            )
                    elif b >= NBLK - OGRP:
                        # final group: per-block 1MB stores so the tail
                        # drains as soon as each block's copies land
                        eng(ring).dma_start(
                            out=OUT[:, b * DOUT : (b + 1) * DOUT],
                            in_=o_sb[:, ob : ob + DOUT],
                        )
                    elif b % OGRP == OGRP - 1:
                        eng(ring).dma_start(
                            out=OUT[:, g * OGRP * DOUT : (g + 1) * OGRP * DOUT],
                            in_=o_sb[:],
                        )

            if not do_compute:
                # DMA-only ablation: consume each X group with a 1-col matmul
                # (so loads stay on the critical path), store a constant o_sb
                o_sb = wpool.tile([128, OGRP * DOUT], BF16, tag="osbc")
                nc.gpsimd.memset(o_sb[:], 0.02)
                for g in range(len(XGS)):
                    acc = po.tile([128, 1], F32, tag="acc")
                    nc.tensor.matmul(
                        acc[:], xg[g][:, 0:128], xg[g][:, 0:1], start=True, stop=True
                    )
                for g in range(NBLK // OGRP):
                    if do_dma:
                        ring = o_rings[g % len(o_rings)]
                        eng(ring).dma_start(
                            out=OUT[:, g * OGRP * DOUT : (g + 1) * OGRP * DOUT],
                            in_=o_sb[:],
                        )
                return

            # 2-deep software pipeline so PE never waits on the ACT/DVE
            # routing chain or the hwT copy.  stB(b+2) is emitted LAST: its
            # exp sem-waits on mm1(b+2), and ACT/DVE process in order, so
            # putting it before the copies/store of block b would head-of-line
            # block the store stream whenever PE is DMA-paced.
            stA(0); stB(0)
            stA(1); stB(1)
            stC(0); stD(0)
            for b in range(NBLK):
                if b + 2 < NBLK:
                    stA(b + 2)
                stEFG(b)
                if b + 1 < NBLK:
                    stC(b + 1); stD(b + 1)
                if b + 2 < NBLK:
                    stB(b + 2)
                # staggered==2: explicit 4-block-aligned stage boundaries so
                # the staggered sem reset never cuts mid-block (the auto
                # instruction-count split misaligns engines and serializes)
                if staggered == 2 and loop and b in (3, 7, 11):
                    tc.stage_boundary()

        if loop:
            assert not (staggered == 2 and body_reps != 1)
            with tc.For_i(0, reps, 1, staggered_reset=bool(staggered)):
                for _ in range(body_reps):
                    emit_all()
        else:
            for _ in range(reps):
                emit_all()

    nc.compile()
    return nc


def _prep_consts(Wr, br, Wt, bt, A, Bw):
    bf = ml_dtypes.bfloat16
    A_all = np.asarray(A, np.float32).transpose(1, 0, 2).reshape(D, ER)  # [d, er]
    Wcat = np.concatenate(
        [np.asarray(Wr, np.float32).T, np.asarray(Wt, np.float32).T], axis=1
    )  # [d, 9]
    AWc_h = np.concatenate(
        [A_all.reshape(NDC, 128, ER), Wcat.reshape(NDC, 128, 9)], axis=2
    )  # [NDC, 128, CW]
    AWc_host = np.ascontiguousarray(
        AWc_h.transpose(1, 0, 2).reshape(128, NDC * CW)
    ).astype(bf)
    biasr = np.zeros((1, CW), np.float32)
    biasr[0, ER : ER + E] = np.asarray(br, np.float32)
    biasr[0, ER + E] = np.float32(np.asarray(bt).reshape(()))
    B_host = (np.asarray(Bw, np.float32).reshape(ER, DOUT) * SCALING).astype(bf)
    idn = np.eye(128, dtype=np.float32).astype(bf)
    wpack_host = np.ascontiguousarray(
        np.concatenate([AWc_host, idn, B_host], axis=1)
    )
    return {
        "AWc": AWc_host,
        "BIASR": biasr.astype(bf),
        "IDN": idn,
        "Bl": B_host,
        "WPACK": wpack_host,
    }


def _prep_x(xs):
    """Per-core shard [NTOK, D] -> [128, NBLK*NDC*BS] bf16,
    partition-contiguous: partition p holds [blk, dc, t] so each X DMA reads
    groups of 8KB-per-block contiguous per partition."""
    arr = (
        np.asarray(xs, np.float32)
        .reshape(NBLK, BS, NDC, 128)
        .transpose(3, 0, 2, 1)  # [p, blk, dc, t]
        .reshape(128, NBLK * NDC * BS)
    )
    return np.ascontiguousarray(arr).astype(ml_dtypes.bfloat16)


def kernel(x, Wr, br, Wt, bt, A, Bw, _trace=False, _trace_kwargs=None):
    # specialize the build on whether the router/threshold biases are zero
    # (they are in the reference); nonzero biases take the with_bias path
    wb = bool(np.any(np.asarray(br)) or np.any(np.asarray(bt)))
    key = f"nc{int(wb)}"
    if key not in _CACHE:
        _CACHE[key] = _build(with_bias=wb, **BEST_KW)
    nc = _CACHE[key]

    consts = _prep_consts(Wr, br, Wt, bt, A, Bw)
    xf = np.asarray(x, np.float32).reshape(N, D)
    in_maps = []
    for c in range(NCORES):
        Xh = _prep_x(xf[c * NTOK : (c + 1) * NTOK])
        in_maps.append({"X": Xh, **consts})

    res = run_bass_kernel_spmd(
        nc,
        in_maps,
        core_ids=list(range(NCORES)),
        trace=_trace,
        **(_trace_kwargs or {}),
    )
    # OUT dram layout is [128, NBLK*DOUT] (partition p = token b*128+p)
    out = np.concatenate(
        [
            np.asarray(res.results[c]["out"], np.float32)
            .reshape(128, NBLK, DOUT)
            .transpose(1, 0, 2)
            .reshape(NTOK, DOUT)
            for c in range(NCORES)
        ],
        axis=0,
    )
    if _trace:
        _CACHE["last_res"] = res
    return out.reshape(B, S, DOUT).astype(np.float32)

